# revision 60
# baseline (speedup 1.0000x reference)
"""Trainium2 Bass kernel for the block-GRU dense MLP (nn_Deter_738734375713).

Strategy: data-parallel over batch across 8 NeuronCores (128 rows/core).
All GEMMs run in bf16 (fp32 PSUM accumulation); norms / gates / GRU math in
fp32. Weights are host-packed into contiguous [128, 512] bf16 tiles and
streamed from HBM. Activations that feed matmuls are kept as transposed
[K=128, M=128] bf16 chunks (the matmul stationary operand); the RMS-norm
row-scale is fused into the transpose as a matmul against diag(rinv), and
the per-feature gain + SiLU + bf16 cast are fused into the PSUM->SBUF copy
on the scalar engine.
"""

import numpy as np
import ml_dtypes

# h0/h1 run fp8 DoubleRow (both operands e4m3, 2 contraction rows per PE
# cell): halves their matmul count. Branches + gate stay bf16-x-e3m4 normal
# mode for accuracy.

import concourse.bass as bass
import concourse.tile as tile
import concourse.mybir as mybir
from concourse import bacc
from concourse.bass_utils import run_bass_kernel_spmd
from concourse.masks import make_identity

BF16 = ml_dtypes.bfloat16
F32 = np.float32
dt = mybir.dt
AF = mybir.ActivationFunctionType
OP = mybir.AluOpType

N_CORES = 8
B = 1024
BL = B // N_CORES            # 128 batch rows per core
DETER, STOCH, ACT_D, HID = 4096, 1024, 128, 1024
BLOCKS, DPB = 8, 512
IN0 = 3 * HID + DPB          # 3584
EPS = 1e-4

# bias row offsets inside the packed brow tensor
B0_OFF = 0
B1_OFF = 1024
B2_OFF = 2048
HB0_OFF = 3072
HB1_OFF = HB0_OFF + 4096     # 7168
GB_OFF = HB1_OFF + 4096      # 11264
BROW_LEN = GB_OFF + 3 * DETER  # 23552

# gain chunk bases inside gT ([128, 96]); rows 88..92 hold per-layer
# c^2*eps values, row 93 holds 1/c_gw (broadcast across partitions)
G0_BASE, G1_BASE, G2_BASE = 0, 8, 16
HG0_BASE, HG1_BASE = 24, 56
EPS_BASE = 88
INVCG_ROW = 93
N_GROWS = 96

# flat fp8 weight stream: 434 [128, 512B] chunks in consumption order.
# DMA'd in variable-size groups: small starters so the first matmul isn't
# gated on a megabyte landing, 16 KB-per-partition steady state (the DMA
# engines are descriptor-latency-bound, so fat descriptors = bandwidth).
# Groups alternate between the two HWDGE rings (sync + scalar).
N_WCHUNKS = 434
WGROUPS = [2048, 2048, 4096] + [8192] * 4 + [16384] * 11 + [2048]
WTOT = sum(WGROUPS)
assert WTOT >= N_WCHUNKS * 512
# ring depth per group size
WG_BUFS = {2048: 2, 4096: 1, 8192: 3, 16384: 4}

# When True, decompose silu(v) = v*sigmoid(v) into sim-supported ops
# (CoreSim lacks the Silu LUT). Hardware builds use the fused Silu.
SIM_SAFE_SILU = False

_CACHE = {}


def _build_nc():
    nc = bacc.Bacc(
        "TRN2",
        target_bir_lowering=False,
        debug=False,
        enable_asserts=False,
        num_devices=N_CORES,
    )

    # ---- DRAM I/O ----
    d = {}
    d["deter"] = nc.dram_tensor("deter", [BL, DETER], dt.float32, kind="ExternalInput").ap()
    d["deterT"] = nc.dram_tensor("deterT", [BL, DETER], dt.bfloat16, kind="ExternalInput").ap()
    d["deterT8"] = nc.dram_tensor("deterT8", [BL, DETER], dt.float8e4, kind="ExternalInput").ap()
    d["stochT"] = nc.dram_tensor("stochT", [BL, STOCH], dt.bfloat16, kind="ExternalInput").ap()
    d["actT"] = nc.dram_tensor("actT", [ACT_D, BL], dt.float32, kind="ExternalInput").ap()
    # all GEMM weights as one flat consumption-ordered fp8 stream, 16 KB per
    # partition per DMA (fp8 e3m4 with per-layer power-of-2 scales; rmsnorm
    # absorbs the scale exactly via a c^2-adjusted eps, the gate layer
    # descales in activations)
    d["wall"] = nc.dram_tensor("wall", [128, WTOT], dt.float8e3, kind="ExternalInput").ap()
    d["gains"] = nc.dram_tensor("gains", [N_GROWS, 128], dt.float32, kind="ExternalInput").ap()
    d["brow"] = nc.dram_tensor("brow", [3, 8192], dt.bfloat16, kind="ExternalInput").ap()
    out = nc.dram_tensor("out", [BL, DETER], dt.float32, kind="ExternalOutput").ap()

    with tile.TileContext(nc) as tc:
        _emit(nc, tc, d, out)

    nc.compile()
    return nc


def _emit(nc, tc, d, out):
    from contextlib import ExitStack

    ctx = ExitStack()
    with ctx:
        io = ctx.enter_context(tc.tile_pool(name="io", bufs=1))
        consts = ctx.enter_context(tc.tile_pool(name="consts", bufs=1))
        wpool = ctx.enter_context(tc.tile_pool(name="w", bufs=4))
        zpool = ctx.enter_context(tc.tile_pool(name="z", bufs=1))
        sqpool = ctx.enter_context(tc.tile_pool(name="sq", bufs=2))
        small = ctx.enter_context(tc.tile_pool(name="small", bufs=1))
        xtpool = ctx.enter_context(tc.tile_pool(name="xt", bufs=1))
        grupool = ctx.enter_context(tc.tile_pool(name="gru", bufs=3))

        # weight stream: 512 B chunks consumed in program order from a ring
        # of variable-size group DMAs, via a chunk FIFO
        wcur = {"g": 0, "base": 0}
        chunk_fifo = []

        def issue_group():
            gi = wcur["g"]
            gsz = WGROUPS[gi]
            t = wpool.tile([128, gsz], dt.float8e3,
                           tag=f"w{gsz}", bufs=WG_BUFS[gsz])
            eng = nc.sync if gi % 2 == 0 else nc.scalar
            eng.dma_start(t[:], d["wall"][:, wcur["base"]:wcur["base"] + gsz])
            wcur["g"] += 1
            wcur["base"] += gsz
            chunk_fifo.extend((t, o) for o in range(0, gsz, 512))

        def stream_chunks(n):
            while len(chunk_fifo) < n:
                issue_group()
            aps = [t[:, o:o + 512] for (t, o) in chunk_fifo[:n]]
            del chunk_fifo[:n]
            return aps

        def stream_pairs(n):
            """Pop 2n chunks as n [128, 2, 512] e4m3 DoubleRow moving APs."""
            while len(chunk_fifo) < 2 * n:
                issue_group()
            ws = []
            for j in range(n):
                (t1, o1), (t2, o2) = chunk_fifo[2 * j], chunk_fifo[2 * j + 1]
                assert t1 is t2 and o2 == o1 + 512, "pair straddles group"
                ws.append(t1[:, o1:o1 + 1024].bitcast(dt.float8e4)
                          .rearrange("p (two n) -> p two n", two=2))
            del chunk_fifo[:2 * n]
            return ws

        # ---- load inputs to SBUF ----
        # deterT (x0 stationary) and the first weight groups gate the first
        # matmuls — issue them ahead of the minor inputs. deter fp32 is only
        # needed by the GRU tail; its DMA is emitted there so the scalar
        # HWDGE ring stays free for weight groups.
        # deterT split in 4 so the first matmuls aren't gated on one big DMA
        deterT_sb = io.tile([128, DETER], dt.bfloat16)
        for _i in range(4):
            nc.sync.dma_start(deterT_sb[:, _i * 1024:(_i + 1) * 1024],
                              d["deterT"][:, _i * 1024:(_i + 1) * 1024])
            if _i == 0:
                issue_group()
                issue_group()
        # bias rows live at partitions 0/32/64 (matmul rhs base-partition
        # constraint), 16 rows of 512 per partition in the free dim
        brow_sb = io.tile([65, 8192], dt.bfloat16)
        for _i in range(3):
            nc.scalar.dma_start(brow_sb[32 * _i:32 * _i + 1, :], d["brow"][_i:_i + 1, :])
        gains_sb = io.tile([N_GROWS, 128], dt.float32)
        nc.sync.dma_start(gains_sb[:], d["gains"][:])
        stochT_sb = io.tile([128, STOCH], dt.bfloat16)
        nc.sync.dma_start(stochT_sb[:], d["stochT"][:])
        actT_sb = io.tile([ACT_D, BL], dt.float32)
        nc.sync.dma_start(actT_sb[:], d["actT"][:])
        deterT8_sb = io.tile([128, DETER], dt.float8e4)
        nc.sync.dma_start(deterT8_sb[:], d["deterT8"][:])
        deter_sb = io.tile([128, DETER], dt.float32)

        def bias_mm(y, b_off):
            """Close the accumulation group with the bias row (K=1 matmul)."""
            r = b_off // 512
            p = 32 * (r // 16)
            nc.tensor.matmul(
                y[:], ones_bf[p:p + 1, :],
                brow_sb[p:p + 1, (r % 16) * 512:(r % 16) * 512 + 512],
                start=False, stop=True)
        ident = consts.tile([128, 128], dt.float32)
        make_identity(nc, ident[:])
        ones_bf = consts.tile([65, 128], dt.bfloat16)
        nc.gpsimd.memset(ones_bf[:], 1.0)
        neg1_b = consts.tile([128, 1], dt.float32)
        nc.gpsimd.memset(neg1_b[:], -1.0)

        with tc.tile_pool(name="psum_tp", bufs=5, space="PSUM") as psum_tp, \
             tc.tile_pool(name="psum_y", bufs=3, space="PSUM") as psum_y:

            # gains: transpose [96,128] -> gT [128, 96]. Emitted lazily at
            # the first finish_layer so the PE FIFO isn't head-blocked on
            # the gains DMA before the x0 GEMMs can run.
            gT = io.tile([128, N_GROWS], dt.float32)
            gT_state = {"done": False}

            def ensure_gT():
                if gT_state["done"]:
                    return
                gT_state["done"] = True
                ps_g = psum_tp.tile([128, 128], dt.float32, tag="tp")
                nc.tensor.transpose(ps_g[:, :N_GROWS], gains_sb[:],
                                    ident[:N_GROWS, :N_GROWS])
                nc.scalar.copy(gT[:], ps_g[:, :N_GROWS])

            # action clip: a = act / max(|act|, 1), in transposed layout, cast bf16
            abs_t = small.tile([ACT_D, BL], dt.float32, tag="acttmp")
            nc.scalar.activation(abs_t[:], actT_sb[:], AF.Abs)
            m_t = small.tile([ACT_D, BL], dt.float32, tag="acttmp2")
            nc.vector.tensor_scalar_max(m_t[:], abs_t[:], 1.0)
            r_t = small.tile([ACT_D, BL], dt.float32, tag="acttmp3")
            nc.vector.reciprocal(r_t[:], m_t[:])
            aT_bf = xtpool.tile([ACT_D, BL], dt.bfloat16, tag="aT")
            nc.vector.tensor_mul(aT_bf[:], actT_sb[:], r_t[:])

            def emit_tile(lhs_list, b_off, ti, zs, partials):
                """GEMM one [128,512] output tile into PSUM, copy to SBUF z,
                and kick off its sum-of-squares partial."""
                y = psum_y.tile([128, 512], dt.float32, tag="y")
                nk = len(lhs_list)
                ws = stream_chunks(nk)
                for k in range(nk):
                    nc.tensor.matmul(y[:], lhs_list[k], ws[k],
                                     start=(k == 0), stop=False)
                bias_mm(y, b_off)
                z = zpool.tile([128, 512], dt.bfloat16, tag="z", bufs=8)
                if ti % 2 == 0:
                    nc.scalar.copy(z[:], y[:])
                else:
                    nc.vector.tensor_copy(z[:], y[:])
                sq = sqpool.tile([128, 512], dt.bfloat16, tag="sq")
                part = small.tile([128, 1], dt.float32, tag="part", bufs=8)
                nc.vector.scalar_tensor_tensor(
                    out=sq[:], in0=z[:], scalar=1.0, in1=z[:],
                    op0=OP.mult, op1=OP.mult, accum_out=part[:])
                zs.append(z)
                partials.append(part)

            def emit_tile_dr(pair_list, b_off, ti, zs, partials):
                """DoubleRow variant: pair_list holds [128, 2, 128] e4m3
                stationary APs; weights stream as [128, 2, 512] pairs."""
                y = psum_y.tile([128, 512], dt.float32, tag="y")
                npair = len(pair_list)
                ws = stream_pairs(npair)
                for k in range(npair):
                    nc.tensor.matmul(y[:], pair_list[k], ws[k],
                                     start=(k == 0), stop=False,
                                     perf_mode=mybir.MatmulPerfMode.DoubleRow)
                bias_mm(y, b_off)
                z = zpool.tile([128, 512], dt.bfloat16, tag="z", bufs=8)
                if ti % 2 == 0:
                    nc.scalar.copy(z[:], y[:])
                else:
                    nc.vector.tensor_copy(z[:], y[:])
                sq = sqpool.tile([128, 512], dt.bfloat16, tag="sq")
                part = small.tile([128, 1], dt.float32, tag="part", bufs=8)
                nc.vector.scalar_tensor_tensor(
                    out=sq[:], in0=z[:], scalar=1.0, in1=z[:],
                    op0=OP.mult, op1=OP.mult, accum_out=part[:])
                zs.append(z)
                partials.append(part)

            def finish_layer(name, zs, partials, D, g_base, n_out_chunks,
                             eps_col, chunk_cb=None, pair_out=False):
                """Combine partials -> 1/rms, then transpose+gain+silu each
                128-chunk (rms scale fused as a matmul against diag(rinv)).
                chunk_cb(ci) is invoked after chunk ci is emitted — used to
                interleave the next layer's GEMMs with this transpose pass."""
                ensure_gT()
                tot = small.tile([128, 1], dt.float32, tag=f"tot_{name}")
                if len(partials) == 1:
                    nc.vector.tensor_copy(tot[:], partials[0][:])
                else:
                    nc.vector.tensor_add(tot[:], partials[0][:], partials[1][:])
                    for p in partials[2:]:
                        nc.vector.tensor_add(tot[:], tot[:], p[:])
                # rinv = rsqrt(tot/D + c^2*eps), all on DVE (quake seed + 2
                # Newton steps; keeps the scalar engine on one act-table set)
                v = small.tile([128, 1], dt.float32, tag=f"v_{name}")
                ec = EPS_BASE + eps_col
                nc.vector.scalar_tensor_tensor(
                    out=v[:], in0=tot[:], scalar=1.0 / D,
                    in1=gT[:, ec:ec + 1],
                    op0=OP.mult, op1=OP.add)
                # seed bits = 0x5F3759DF - (bits>>1), built overflow-free:
                # (bits>>1) xor 0x7FFFFFFF == 0x7FFFFFFF - (bits>>1) exactly,
                # then subtract 0x20C8A620 (fp32 ALU path, values < 2^31)
                yq = small.tile([128, 1], dt.float32, tag=f"yq_{name}")
                nc.vector.tensor_scalar(
                    out=yq[:].bitcast(dt.uint32), in0=v[:].bitcast(dt.uint32),
                    scalar1=1, scalar2=0x7FFFFFFF,
                    op0=OP.logical_shift_right, op1=OP.bitwise_xor)
                nc.vector.tensor_scalar_sub(
                    yq[:].bitcast(dt.uint32), yq[:].bitcast(dt.uint32),
                    0x20C8A620)
                rinv = yq
                for _it in range(1):
                    a = small.tile([128, 1], dt.float32, tag=f"nra_{name}{_it}")
                    nc.vector.tensor_mul(a[:], v[:], rinv[:])
                    nc.vector.tensor_mul(a[:], a[:], rinv[:])
                    nc.vector.tensor_scalar(
                        out=a[:], in0=a[:], scalar1=-0.5, scalar2=1.5,
                        op0=OP.mult, op1=OP.add)
                    r2 = small.tile([128, 1], dt.float32, tag=f"nrr_{name}{_it}")
                    nc.vector.tensor_mul(r2[:], rinv[:], a[:])
                    rinv = r2
                diag = small.tile([128, 128], dt.bfloat16, tag=f"diag_{name}")
                nc.vector.tensor_scalar_mul(diag[:], ident[:], rinv[:])
                chunks = []
                cur_pair = None
                for ci in range(n_out_chunks):
                    ti, c4 = divmod(ci, 4)
                    pt = psum_tp.tile([128, 128], dt.float32, tag="tp")
                    nc.tensor.matmul(pt[:], zs[ti][:, c4 * 128:(c4 + 1) * 128],
                                     diag[:], start=True, stop=True)
                    if pair_out:
                        if ci % 2 == 0:
                            cur_pair = xtpool.tile(
                                [128, 256], dt.float8e4, tag=f"xp_{name}",
                                bufs=n_out_chunks // 2)
                        dst = cur_pair[:, (ci % 2) * 128:(ci % 2) * 128 + 128]
                    else:
                        xt = xtpool.tile([128, 128], dt.bfloat16,
                                         tag=f"xt_{name}", bufs=n_out_chunks)
                        dst = xt[:]
                    gsl = gT[:, g_base + ci:g_base + ci + 1]
                    if SIM_SAFE_SILU:
                        sg = sqpool.tile([128, 128], dt.float32, tag="simsg")
                        nc.scalar.activation(sg[:], pt[:], AF.Sigmoid, scale=gsl)
                        vv = sqpool.tile([128, 128], dt.float32, tag="simv")
                        nc.scalar.activation(vv[:], pt[:], AF.Copy, scale=gsl)
                        nc.vector.tensor_mul(dst, sg[:], vv[:])
                    else:
                        nc.scalar.activation(dst, pt[:], AF.Silu, scale=gsl)
                    if pair_out:
                        if ci % 2 == 1:
                            chunks.append(cur_pair)
                    else:
                        chunks.append(xt)
                    if chunk_cb is not None:
                        chunk_cb(ci, chunks)
                return chunks

            def as_pair(ap):
                return ap.rearrange("p (two m) -> p two m", two=2)

            dT = [deterT_sb[:, c * 128:(c + 1) * 128] for c in range(32)]
            sT = [stochT_sb[:, c * 128:(c + 1) * 128] for c in range(8)]

            # input branches: emit ALL three branches' GEMMs first, then the
            # finish passes — the PE FIFO runs x1/x2 GEMMs while x0's norm
            # chain (DVE rsqrt) resolves, instead of stalling behind the
            # x0 transposes.
            bzs = {k: ([], []) for k in ("x0", "x1", "x2")}
            for ti, n in enumerate(range(2)):
                emit_tile(dT, B0_OFF + n * 512, ti, *bzs["x0"])
            for ti, n in enumerate(range(2)):
                emit_tile(sT, B1_OFF + n * 512, ti, *bzs["x1"])
            for ti, n in enumerate(range(2)):
                emit_tile([aT_bf[:]], B2_OFF + n * 512, ti, *bzs["x2"])
            x0P = finish_layer("x0", *bzs["x0"], HID, G0_BASE, 8, 0,
                               pair_out=True)
            x1P = finish_layer("x1", *bzs["x1"], HID, G1_BASE, 8, 1,
                               pair_out=True)
            x2P = finish_layer("x2", *bzs["x2"], HID, G2_BASE, 8, 2,
                               pair_out=True)

            xPairs = [as_pair(t[:]) for t in (x0P + x1P + x2P)]

            # hidden 0 (DoubleRow): per block, in = [deter_g, x] (14 pairs)
            h0_zs, h0_parts = [], []
            for g in range(BLOCKS):
                dpairs = [as_pair(deterT8_sb[:, g * 512 + j * 256:
                                             g * 512 + (j + 1) * 256])
                          for j in range(2)]
                emit_tile_dr(dpairs + xPairs,
                             HB0_OFF + g * 512, g, h0_zs, h0_parts)


            # h0 norm+transpose pass with hidden-1 GEMMs interleaved: as soon
            # as block g's 4 h0n chunks (2 pair tiles) exist, emit h1 tile
            # g's DoubleRow matmuls.
            h1_zs, h1_parts = [], []

            def h0_cb(ci, chunks):
                if ci % 4 == 3:
                    g = ci // 4
                    emit_tile_dr([as_pair(chunks[2 * g][:]),
                                  as_pair(chunks[2 * g + 1][:])],
                                 HB1_OFF + g * 512, g, h1_zs, h1_parts)

            finish_layer("h0", h0_zs, h0_parts, DETER, HG0_BASE, 32,
                         3, chunk_cb=h0_cb, pair_out=True)
            h1nT = finish_layer("h1", h1_zs, h1_parts, DETER, HG1_BASE, 32, 4)

        # ---- gate layer + GRU (no norm) ----
        # flush the remaining weight groups onto the rings first so deter's
        # big DMA doesn't head-block the gate weights on the scalar ring
        while wcur["g"] < len(WGROUPS):
            issue_group()
        nc.scalar.dma_start(deter_sb[:], d["deter"][:])
        with tc.tile_pool(name="psum_g", bufs=8, space="PSUM") as psum_g:
            for g in range(BLOCKS):
                ys = []
                for ntile in range(3):
                    y = psum_g.tile([128, 512], dt.float32, tag="gy")
                    b_off = GB_OFF + g * 1536 + ntile * 512
                    ws = stream_chunks(4)
                    for k in range(4):
                        nc.tensor.matmul(y[:], h1nT[4 * g + k][:],
                                         ws[k],
                                         start=(k == 0), stop=False)
                    bias_mm(y, b_off)
                    ys.append(y)
                y_r, y_c, y_u = ys
                dslice = deter_sb[:, g * 512:(g + 1) * 512]
                inv_cg = gT[:, INVCG_ROW:INVCG_ROW + 1]

                reset = grupool.tile([128, 512], dt.float32, tag="reset")
                nc.scalar.activation(reset[:], y_r[:], AF.Sigmoid, scale=inv_cg)
                nc.vector.tensor_mul(reset[:], reset[:], y_c[:])
                cand = grupool.tile([128, 512], dt.float32, tag="cand")
                nc.scalar.activation(cand[:], reset[:], AF.Tanh, scale=inv_cg)
                upd = grupool.tile([128, 512], dt.float32, tag="upd")
                nc.scalar.activation(upd[:], y_u[:], AF.Sigmoid, bias=neg1_b[:],
                                     scale=inv_cg)
                acc = grupool.tile([128, 512], dt.float32, tag="acc")
                nc.vector.tensor_sub(acc[:], cand[:], dslice)
                nc.vector.tensor_mul(acc[:], upd[:], acc[:])
                nc.vector.tensor_add(acc[:], acc[:], dslice)
                nc.scalar.dma_start(out[:, g * 512:(g + 1) * 512], acc[:])


# ---------------- host side ----------------

E3 = ml_dtypes.float8_e3m4
E4 = ml_dtypes.float8_e4m3


def _fp8_scale(w, mx=15.5):
    """Power-of-2 scale landing absmax(w) at ~75% of the fp8 max."""
    absmax = float(np.abs(w).max())
    if absmax == 0.0:
        return 1.0
    return float(2.0 ** np.floor(np.log2(mx * 0.75 / absmax)))


def _gemm_chunks(w, c, nt=None):
    """w [K, N] f32 -> list of [128, 512] fp32 chunks (scaled by c) in
    emission order: n-tile major, k-chunk minor."""
    K, N = w.shape
    nt = N // 512 if nt is None else nt
    kc = K // 128
    out = []
    for n in range(nt):
        for k in range(kc):
            out.append(w[k * 128:(k + 1) * 128, n * 512:(n + 1) * 512] * c)
    return out


def _sbuf_image_T(x, nchunks, dtype=BF16):
    """x [BL, D] -> [128, D] image where S[p, c*128+m] = x[m, 128c+p]."""
    BLl, D = x.shape
    assert D == nchunks * 128 and BLl == BL
    t = x.T.reshape(nchunks, 128, BLl).transpose(1, 0, 2)
    return np.ascontiguousarray(t.reshape(128, D)).astype(dtype)


def _prep_shared(inp):
    """Pack weights/biases/gains (shared across cores). Weights go to fp8
    e3m4 with per-layer power-of-2 scales c; biases are scaled to match, and
    the scale is undone on device (c^2-adjusted eps for normed layers, 1/c
    activation scale for the gate layer)."""
    sh = {}
    c0 = _fp8_scale(inp["w0"])
    c1 = _fp8_scale(inp["w1"])
    c2 = _fp8_scale(inp["w2"])
    ch0 = _fp8_scale(inp["hw0"], mx=240.0)
    ch1 = _fp8_scale(inp["hw1"], mx=240.0)
    cg = _fp8_scale(inp["gw"])

    # flat weight stream in exact consumption order (see _emit); h0/h1
    # regions are e4m3 bytes (DoubleRow), the rest e3m4
    chunks = []
    chunks += [(c, E3) for c in _gemm_chunks(inp["w0"], c0)]
    chunks += [(c, E3) for c in _gemm_chunks(inp["w1"], c1)]
    chunks += [(c, E3) for c in _gemm_chunks(inp["w2"], c2)]
    for g in range(BLOCKS):
        chunks += [(c, E4) for c in _gemm_chunks(inp["hw0"][g], ch0)]
    for g in range(BLOCKS):
        chunks += [(c, E4) for c in _gemm_chunks(inp["hw1"][g], ch1)]
    for g in range(BLOCKS):
        chunks += [(c, E3) for c in _gemm_chunks(inp["gw"][g], cg)]
    assert len(chunks) == N_WCHUNKS
    flat = np.stack([np.asarray(c, F32).astype(dtp).view(np.uint8)
                     for c, dtp in chunks])          # [434, 128, 512] bytes
    flat = flat.transpose(1, 0, 2).reshape(128, N_WCHUNKS * 512).view(E3)
    pad = WTOT - N_WCHUNKS * 512
    sh["wall"] = np.ascontiguousarray(
        np.concatenate([flat, np.zeros((128, pad), E3)], axis=1))

    grows = np.concatenate(
        [inp[k].reshape(-1, 128) for k in ("g0", "g1", "g2", "hg0", "hg1")],
        axis=0).astype(F32)
    extra = np.zeros((N_GROWS - 88, 128), F32)
    for i, c in enumerate((c0, c1, c2, ch0, ch1)):
        extra[i, :] = c * c * EPS
    extra[INVCG_ROW - 88, :] = 1.0 / cg
    sh["gains"] = np.concatenate([grows, extra], axis=0)
    _b = np.concatenate(
        [inp["b0"] * c0, inp["b1"] * c1, inp["b2"] * c2,
         inp["hb0"] * ch0, inp["hb1"] * ch1, inp["gb"] * cg])
    _b = np.concatenate([_b, np.zeros(3 * 8192 - BROW_LEN, _b.dtype)])
    sh["brow"] = _b.reshape(3, 8192).astype(BF16)
    return sh


def kernel(**inputs):
    inputs = {k: np.asarray(v) for k, v in inputs.items()}
    stoch = inputs["stoch"].reshape(B, -1).astype(F32)
    deter = inputs["deter"].astype(F32)
    action = inputs["action"].astype(F32)
    assert deter.shape == (B, DETER) and stoch.shape == (B, STOCH)
    assert action.shape == (B, ACT_D)

    if "nc" not in _CACHE:
        _CACHE["nc"] = _build_nc()
    nc = _CACHE["nc"]

    sh = _prep_shared(inputs)

    in_maps = []
    for c in range(N_CORES):
        s = slice(c * BL, (c + 1) * BL)
        m = dict(sh)
        m["deter"] = np.ascontiguousarray(deter[s])
        m["deterT"] = _sbuf_image_T(deter[s], 32)
        m["deterT8"] = _sbuf_image_T(deter[s], 32, dtype=E4)
        m["stochT"] = _sbuf_image_T(stoch[s], 8)
        m["actT"] = np.ascontiguousarray(action[s].T).astype(F32)
        in_maps.append(m)

    res = run_bass_kernel_spmd(nc, in_maps, core_ids=list(range(N_CORES)))
    return np.concatenate([res.results[c]["out"] for c in range(N_CORES)], axis=0)



# revision 65
# speedup vs baseline: 1.0037x; 1.0037x over previous
"""Trainium2 Bass kernel for the block-GRU dense MLP (nn_Deter_738734375713).

Strategy: data-parallel over batch across 8 NeuronCores (128 rows/core).
All GEMMs run in bf16 (fp32 PSUM accumulation); norms / gates / GRU math in
fp32. Weights are host-packed into contiguous [128, 512] bf16 tiles and
streamed from HBM. Activations that feed matmuls are kept as transposed
[K=128, M=128] bf16 chunks (the matmul stationary operand); the RMS-norm
row-scale is fused into the transpose as a matmul against diag(rinv), and
the per-feature gain + SiLU + bf16 cast are fused into the PSUM->SBUF copy
on the scalar engine.
"""

import numpy as np
import ml_dtypes

# h0/h1 run fp8 DoubleRow (both operands e4m3, 2 contraction rows per PE
# cell): halves their matmul count. Branches + gate stay bf16-x-e3m4 normal
# mode for accuracy.

import concourse.bass as bass
import concourse.tile as tile
import concourse.mybir as mybir
from concourse import bacc
from concourse.bass_utils import run_bass_kernel_spmd
from concourse.masks import make_identity

BF16 = ml_dtypes.bfloat16
F32 = np.float32
dt = mybir.dt
AF = mybir.ActivationFunctionType
OP = mybir.AluOpType

N_CORES = 8
B = 1024
BL = B // N_CORES            # 128 batch rows per core
DETER, STOCH, ACT_D, HID = 4096, 1024, 128, 1024
BLOCKS, DPB = 8, 512
IN0 = 3 * HID + DPB          # 3584
EPS = 1e-4

# bias row offsets inside the packed brow tensor
B0_OFF = 0
B1_OFF = 1024
B2_OFF = 2048
HB0_OFF = 3072
HB1_OFF = HB0_OFF + 4096     # 7168
GB_OFF = HB1_OFF + 4096      # 11264
BROW_LEN = GB_OFF + 3 * DETER  # 23552

# gain chunk bases inside gT ([128, 96]); rows 88..92 hold per-layer
# c^2*eps values, row 93 holds 1/c_gw (broadcast across partitions)
G0_BASE, G1_BASE, G2_BASE = 0, 8, 16
HG0_BASE, HG1_BASE = 24, 56
EPS_BASE = 88
INVCG_ROW = 93
N_GROWS = 96

# flat fp8 weight stream: 434 [128, 512B] chunks in consumption order.
# DMA'd in variable-size groups: small starters so the first matmul isn't
# gated on a megabyte landing, 16 KB-per-partition steady state (the DMA
# engines are descriptor-latency-bound, so fat descriptors = bandwidth).
# Groups alternate between the two HWDGE rings (sync + scalar).
N_WCHUNKS = 434
WGROUPS = [2048, 2048, 4096] + [8192] * 4 + [16384] * 11 + [2048]
WTOT = sum(WGROUPS)
assert WTOT >= N_WCHUNKS * 512
# ring depth per group size; the 16 KB steady-state ring is deep on purpose:
# the DMA engines deliver ~430 GB/s aggregate while the h0 DoubleRow phase
# consumes ~600 GB/s, so ~4 MB (32 KB/partition) must be prefetched during
# the input-branch phase to bridge the deficit
WG_BUFS = {2048: 2, 4096: 1, 8192: 2, 16384: 6}

# When True, decompose silu(v) = v*sigmoid(v) into sim-supported ops
# (CoreSim lacks the Silu LUT). Hardware builds use the fused Silu.
SIM_SAFE_SILU = False

_CACHE = {}


def _build_nc():
    nc = bacc.Bacc(
        "TRN2",
        target_bir_lowering=False,
        debug=False,
        enable_asserts=False,
        num_devices=N_CORES,
    )

    # ---- DRAM I/O ----
    d = {}
    d["deter"] = nc.dram_tensor("deter", [BL, DETER], dt.float32, kind="ExternalInput").ap()
    d["deterT"] = nc.dram_tensor("deterT", [BL, DETER], dt.bfloat16, kind="ExternalInput").ap()
    d["deterT8"] = nc.dram_tensor("deterT8", [BL, DETER], dt.float8e4, kind="ExternalInput").ap()
    d["stochT"] = nc.dram_tensor("stochT", [BL, STOCH], dt.bfloat16, kind="ExternalInput").ap()
    d["actT"] = nc.dram_tensor("actT", [ACT_D, BL], dt.float32, kind="ExternalInput").ap()
    # all GEMM weights as one flat consumption-ordered fp8 stream, 16 KB per
    # partition per DMA (fp8 e3m4 with per-layer power-of-2 scales; rmsnorm
    # absorbs the scale exactly via a c^2-adjusted eps, the gate layer
    # descales in activations)
    d["wall"] = nc.dram_tensor("wall", [128, WTOT], dt.float8e3, kind="ExternalInput").ap()
    d["gains"] = nc.dram_tensor("gains", [N_GROWS, 128], dt.float32, kind="ExternalInput").ap()
    d["brow"] = nc.dram_tensor("brow", [3, 8192], dt.bfloat16, kind="ExternalInput").ap()
    out = nc.dram_tensor("out", [BL, DETER], dt.float32, kind="ExternalOutput").ap()

    with tile.TileContext(nc) as tc:
        _emit(nc, tc, d, out)

    nc.compile()
    return nc


def _emit(nc, tc, d, out):
    from contextlib import ExitStack

    ctx = ExitStack()
    with ctx:
        io = ctx.enter_context(tc.tile_pool(name="io", bufs=1))
        consts = ctx.enter_context(tc.tile_pool(name="consts", bufs=1))
        wpool = ctx.enter_context(tc.tile_pool(name="w", bufs=4))
        zpool = ctx.enter_context(tc.tile_pool(name="z", bufs=1))
        sqpool = ctx.enter_context(tc.tile_pool(name="sq", bufs=2))
        small = ctx.enter_context(tc.tile_pool(name="small", bufs=1))
        xtpool = ctx.enter_context(tc.tile_pool(name="xt", bufs=1))
        grupool = ctx.enter_context(tc.tile_pool(name="gru", bufs=2))

        # weight stream: 512 B chunks consumed in program order from a ring
        # of variable-size group DMAs, via a chunk FIFO
        wcur = {"g": 0, "base": 0}
        chunk_fifo = []

        def issue_group():
            gi = wcur["g"]
            gsz = WGROUPS[gi]
            t = wpool.tile([128, gsz], dt.float8e3,
                           tag=f"w{gsz}", bufs=WG_BUFS[gsz])
            eng = nc.sync if gi % 2 == 0 else nc.scalar
            eng.dma_start(t[:], d["wall"][:, wcur["base"]:wcur["base"] + gsz])
            wcur["g"] += 1
            wcur["base"] += gsz
            chunk_fifo.extend((t, o) for o in range(0, gsz, 512))

        def stream_chunks(n):
            while len(chunk_fifo) < n:
                issue_group()
            aps = [t[:, o:o + 512] for (t, o) in chunk_fifo[:n]]
            del chunk_fifo[:n]
            return aps

        def stream_pairs(n):
            """Pop 2n chunks as n [128, 2, 512] e4m3 DoubleRow moving APs."""
            while len(chunk_fifo) < 2 * n:
                issue_group()
            ws = []
            for j in range(n):
                (t1, o1), (t2, o2) = chunk_fifo[2 * j], chunk_fifo[2 * j + 1]
                assert t1 is t2 and o2 == o1 + 512, "pair straddles group"
                ws.append(t1[:, o1:o1 + 1024].bitcast(dt.float8e4)
                          .rearrange("p (two n) -> p two n", two=2))
            del chunk_fifo[:2 * n]
            return ws

        # ---- load inputs to SBUF ----
        # deterT (x0 stationary) and the first weight groups gate the first
        # matmuls — issue them ahead of the minor inputs. deter fp32 is only
        # needed by the GRU tail; its DMA is emitted there so the scalar
        # HWDGE ring stays free for weight groups.
        # deterT split in 4 so the first matmuls aren't gated on one big DMA
        deterT_sb = io.tile([128, DETER], dt.bfloat16)
        for _i in range(4):
            nc.sync.dma_start(deterT_sb[:, _i * 1024:(_i + 1) * 1024],
                              d["deterT"][:, _i * 1024:(_i + 1) * 1024])
            if _i == 0:
                issue_group()
                issue_group()
        # bias rows live at partitions 0/32/64 (matmul rhs base-partition
        # constraint), 16 rows of 512 per partition in the free dim
        brow_sb = io.tile([65, 8192], dt.bfloat16)
        for _i in range(3):
            nc.scalar.dma_start(brow_sb[32 * _i:32 * _i + 1, :], d["brow"][_i:_i + 1, :])
        gains_sb = io.tile([N_GROWS, 128], dt.float32)
        nc.sync.dma_start(gains_sb[:], d["gains"][:])
        stochT_sb = io.tile([128, STOCH], dt.bfloat16)
        nc.scalar.dma_start(stochT_sb[:], d["stochT"][:])
        actT_sb = io.tile([ACT_D, BL], dt.float32)
        nc.sync.dma_start(actT_sb[:], d["actT"][:])
        deterT8_sb = io.tile([128, DETER], dt.float8e4)
        nc.scalar.dma_start(deterT8_sb[:], d["deterT8"][:])

        def bias_mm(y, b_off):
            """Close the accumulation group with the bias row (K=1 matmul)."""
            r = b_off // 512
            p = 32 * (r // 16)
            nc.tensor.matmul(
                y[:], ones_bf[p:p + 1, :],
                brow_sb[p:p + 1, (r % 16) * 512:(r % 16) * 512 + 512],
                start=False, stop=True)
        ident = consts.tile([128, 128], dt.float32)
        make_identity(nc, ident[:])
        ones_bf = consts.tile([65, 128], dt.bfloat16)
        nc.gpsimd.memset(ones_bf[:], 1.0)
        neg1_b = consts.tile([128, 1], dt.float32)
        nc.gpsimd.memset(neg1_b[:], -1.0)

        with tc.tile_pool(name="psum_tp", bufs=5, space="PSUM") as psum_tp, \
             tc.tile_pool(name="psum_y", bufs=3, space="PSUM") as psum_y:

            # gains: transpose [96,128] -> gT [128, 96]. Emitted lazily at
            # the first finish_layer so the PE FIFO isn't head-blocked on
            # the gains DMA before the x0 GEMMs can run.
            gT = io.tile([128, N_GROWS], dt.float32)
            gT_state = {"done": False}

            def ensure_gT():
                if gT_state["done"]:
                    return
                gT_state["done"] = True
                ps_g = psum_tp.tile([128, 128], dt.float32, tag="tp")
                nc.tensor.transpose(ps_g[:, :N_GROWS], gains_sb[:],
                                    ident[:N_GROWS, :N_GROWS])
                nc.scalar.copy(gT[:], ps_g[:, :N_GROWS])

            # action clip: a = act / max(|act|, 1), in transposed layout, cast bf16
            abs_t = small.tile([ACT_D, BL], dt.float32, tag="acttmp")
            nc.scalar.activation(abs_t[:], actT_sb[:], AF.Abs)
            m_t = small.tile([ACT_D, BL], dt.float32, tag="acttmp2")
            nc.vector.tensor_scalar_max(m_t[:], abs_t[:], 1.0)
            r_t = small.tile([ACT_D, BL], dt.float32, tag="acttmp3")
            nc.vector.reciprocal(r_t[:], m_t[:])
            aT_bf = xtpool.tile([ACT_D, BL], dt.bfloat16, tag="aT")
            nc.vector.tensor_mul(aT_bf[:], actT_sb[:], r_t[:])

            def emit_tile(lhs_list, b_off, ti, zs, partials):
                """GEMM one [128,512] output tile into PSUM, copy to SBUF z,
                and kick off its sum-of-squares partial."""
                y = psum_y.tile([128, 512], dt.float32, tag="y")
                nk = len(lhs_list)
                ws = stream_chunks(nk)
                for k in range(nk):
                    nc.tensor.matmul(y[:], lhs_list[k], ws[k],
                                     start=(k == 0), stop=False)
                bias_mm(y, b_off)
                z = zpool.tile([128, 512], dt.bfloat16, tag="z", bufs=8)
                if ti % 2 == 0:
                    nc.scalar.copy(z[:], y[:])
                else:
                    nc.vector.tensor_copy(z[:], y[:])
                sq = sqpool.tile([128, 512], dt.bfloat16, tag="sq")
                part = small.tile([128, 1], dt.float32, tag="part", bufs=8)
                nc.vector.scalar_tensor_tensor(
                    out=sq[:], in0=z[:], scalar=1.0, in1=z[:],
                    op0=OP.mult, op1=OP.mult, accum_out=part[:])
                zs.append(z)
                partials.append(part)

            def emit_tile_dr(pair_list, b_off, ti, zs, partials):
                """DoubleRow variant: pair_list holds [128, 2, 128] e4m3
                stationary APs; weights stream as [128, 2, 512] pairs."""
                y = psum_y.tile([128, 512], dt.float32, tag="y")
                npair = len(pair_list)
                ws = stream_pairs(npair)
                for k in range(npair):
                    nc.tensor.matmul(y[:], pair_list[k], ws[k],
                                     start=(k == 0), stop=False,
                                     perf_mode=mybir.MatmulPerfMode.DoubleRow)
                bias_mm(y, b_off)
                z = zpool.tile([128, 512], dt.bfloat16, tag="z", bufs=8)
                if ti % 2 == 0:
                    nc.scalar.copy(z[:], y[:])
                else:
                    nc.vector.tensor_copy(z[:], y[:])
                sq = sqpool.tile([128, 512], dt.bfloat16, tag="sq")
                part = small.tile([128, 1], dt.float32, tag="part", bufs=8)
                nc.vector.scalar_tensor_tensor(
                    out=sq[:], in0=z[:], scalar=1.0, in1=z[:],
                    op0=OP.mult, op1=OP.mult, accum_out=part[:])
                zs.append(z)
                partials.append(part)

            def finish_layer(name, zs, partials, D, g_base, n_out_chunks,
                             eps_col, chunk_cb=None, pair_out=False):
                """Combine partials -> 1/rms, then transpose+gain+silu each
                128-chunk (rms scale fused as a matmul against diag(rinv)).
                chunk_cb(ci) is invoked after chunk ci is emitted — used to
                interleave the next layer's GEMMs with this transpose pass."""
                ensure_gT()
                tot = small.tile([128, 1], dt.float32, tag=f"tot_{name}")
                if len(partials) == 1:
                    nc.vector.tensor_copy(tot[:], partials[0][:])
                else:
                    nc.vector.tensor_add(tot[:], partials[0][:], partials[1][:])
                    for p in partials[2:]:
                        nc.vector.tensor_add(tot[:], tot[:], p[:])
                # rinv = rsqrt(tot/D + c^2*eps), all on DVE (quake seed + 2
                # Newton steps; keeps the scalar engine on one act-table set)
                v = small.tile([128, 1], dt.float32, tag=f"v_{name}")
                ec = EPS_BASE + eps_col
                nc.vector.scalar_tensor_tensor(
                    out=v[:], in0=tot[:], scalar=1.0 / D,
                    in1=gT[:, ec:ec + 1],
                    op0=OP.mult, op1=OP.add)
                # seed bits = 0x5F3759DF - (bits>>1), built overflow-free:
                # (bits>>1) xor 0x7FFFFFFF == 0x7FFFFFFF - (bits>>1) exactly,
                # then subtract 0x20C8A620 (fp32 ALU path, values < 2^31)
                yq = small.tile([128, 1], dt.float32, tag=f"yq_{name}")
                nc.vector.tensor_scalar(
                    out=yq[:].bitcast(dt.uint32), in0=v[:].bitcast(dt.uint32),
                    scalar1=1, scalar2=0x7FFFFFFF,
                    op0=OP.logical_shift_right, op1=OP.bitwise_xor)
                nc.vector.tensor_scalar_sub(
                    yq[:].bitcast(dt.uint32), yq[:].bitcast(dt.uint32),
                    0x20C8A620)
                rinv = yq
                for _it in range(1):
                    a = small.tile([128, 1], dt.float32, tag=f"nra_{name}{_it}")
                    nc.vector.tensor_mul(a[:], v[:], rinv[:])
                    nc.vector.tensor_mul(a[:], a[:], rinv[:])
                    nc.vector.tensor_scalar(
                        out=a[:], in0=a[:], scalar1=-0.5, scalar2=1.5,
                        op0=OP.mult, op1=OP.add)
                    r2 = small.tile([128, 1], dt.float32, tag=f"nrr_{name}{_it}")
                    nc.vector.tensor_mul(r2[:], rinv[:], a[:])
                    rinv = r2
                diag = small.tile([128, 128], dt.bfloat16, tag=f"diag_{name}")
                nc.vector.tensor_scalar_mul(diag[:], ident[:], rinv[:])
                chunks = []
                cur_pair = None
                for ci in range(n_out_chunks):
                    ti, c4 = divmod(ci, 4)
                    pt = psum_tp.tile([128, 128], dt.float32, tag="tp")
                    nc.tensor.matmul(pt[:], zs[ti][:, c4 * 128:(c4 + 1) * 128],
                                     diag[:], start=True, stop=True)
                    if pair_out:
                        if ci % 2 == 0:
                            cur_pair = xtpool.tile(
                                [128, 256], dt.float8e4, tag=f"xp_{name}",
                                bufs=n_out_chunks // 2)
                        dst = cur_pair[:, (ci % 2) * 128:(ci % 2) * 128 + 128]
                    else:
                        xt = xtpool.tile([128, 128], dt.bfloat16,
                                         tag=f"xt_{name}", bufs=n_out_chunks)
                        dst = xt[:]
                    gsl = gT[:, g_base + ci:g_base + ci + 1]
                    if SIM_SAFE_SILU:
                        sg = sqpool.tile([128, 128], dt.float32, tag="simsg")
                        nc.scalar.activation(sg[:], pt[:], AF.Sigmoid, scale=gsl)
                        vv = sqpool.tile([128, 128], dt.float32, tag="simv")
                        nc.scalar.activation(vv[:], pt[:], AF.Copy, scale=gsl)
                        nc.vector.tensor_mul(dst, sg[:], vv[:])
                    else:
                        nc.scalar.activation(dst, pt[:], AF.Silu, scale=gsl)
                    if pair_out:
                        if ci % 2 == 1:
                            chunks.append(cur_pair)
                    else:
                        chunks.append(xt)
                    if chunk_cb is not None:
                        chunk_cb(ci, chunks)
                return chunks

            def as_pair(ap):
                return ap.rearrange("p (two m) -> p two m", two=2)

            dT = [deterT_sb[:, c * 128:(c + 1) * 128] for c in range(32)]
            sT = [stochT_sb[:, c * 128:(c + 1) * 128] for c in range(8)]

            # input branches: emit ALL three branches' GEMMs first, then the
            # finish passes — the PE FIFO runs x1/x2 GEMMs while x0's norm
            # chain (DVE rsqrt) resolves, instead of stalling behind the
            # x0 transposes.
            bzs = {k: ([], []) for k in ("x0", "x1", "x2")}
            for ti, n in enumerate(range(2)):
                emit_tile(dT, B0_OFF + n * 512, ti, *bzs["x0"])
            for ti, n in enumerate(range(2)):
                emit_tile(sT, B1_OFF + n * 512, ti, *bzs["x1"])
            for ti, n in enumerate(range(2)):
                emit_tile([aT_bf[:]], B2_OFF + n * 512, ti, *bzs["x2"])
            x0P = finish_layer("x0", *bzs["x0"], HID, G0_BASE, 8, 0,
                               pair_out=True)
            x1P = finish_layer("x1", *bzs["x1"], HID, G1_BASE, 8, 1,
                               pair_out=True)
            x2P = finish_layer("x2", *bzs["x2"], HID, G2_BASE, 8, 2,
                               pair_out=True)

            xPairs = [as_pair(t[:]) for t in (x0P + x1P + x2P)]

            # hidden 0 (DoubleRow): per block, in = [deter_g, x] (14 pairs)
            h0_zs, h0_parts = [], []
            for g in range(BLOCKS):
                dpairs = [as_pair(deterT8_sb[:, g * 512 + j * 256:
                                             g * 512 + (j + 1) * 256])
                          for j in range(2)]
                emit_tile_dr(dpairs + xPairs,
                             HB0_OFF + g * 512, g, h0_zs, h0_parts)


            # h0 norm+transpose pass with hidden-1 GEMMs interleaved: as soon
            # as block g's 4 h0n chunks (2 pair tiles) exist, emit h1 tile
            # g's DoubleRow matmuls.
            h1_zs, h1_parts = [], []

            def h0_cb(ci, chunks):
                if ci % 4 == 3:
                    g = ci // 4
                    emit_tile_dr([as_pair(chunks[2 * g][:]),
                                  as_pair(chunks[2 * g + 1][:])],
                                 HB1_OFF + g * 512, g, h1_zs, h1_parts)

            finish_layer("h0", h0_zs, h0_parts, DETER, HG0_BASE, 32,
                         3, chunk_cb=h0_cb, pair_out=True)
            h1nT = finish_layer("h1", h1_zs, h1_parts, DETER, HG1_BASE, 32, 4)

        # ---- gate layer + GRU (no norm) ----
        with tc.tile_pool(name="psum_g", bufs=8, space="PSUM") as psum_g:
            for g in range(BLOCKS):
                dsl = grupool.tile([128, 512], dt.float32, tag="dsl")
                nc.scalar.dma_start(dsl[:], d["deter"][:, g * 512:(g + 1) * 512])
                ys = []
                for ntile in range(3):
                    y = psum_g.tile([128, 512], dt.float32, tag="gy")
                    b_off = GB_OFF + g * 1536 + ntile * 512
                    ws = stream_chunks(4)
                    for k in range(4):
                        nc.tensor.matmul(y[:], h1nT[4 * g + k][:],
                                         ws[k],
                                         start=(k == 0), stop=False)
                    bias_mm(y, b_off)
                    ys.append(y)
                y_r, y_c, y_u = ys
                dslice = dsl[:]
                inv_cg = gT[:, INVCG_ROW:INVCG_ROW + 1]

                reset = grupool.tile([128, 512], dt.float32, tag="reset")
                nc.scalar.activation(reset[:], y_r[:], AF.Sigmoid, scale=inv_cg)
                nc.vector.tensor_mul(reset[:], reset[:], y_c[:])
                cand = grupool.tile([128, 512], dt.float32, tag="cand")
                nc.scalar.activation(cand[:], reset[:], AF.Tanh, scale=inv_cg)
                upd = grupool.tile([128, 512], dt.float32, tag="upd")
                nc.scalar.activation(upd[:], y_u[:], AF.Sigmoid, bias=neg1_b[:],
                                     scale=inv_cg)
                acc = grupool.tile([128, 512], dt.float32, tag="acc")
                nc.vector.tensor_sub(acc[:], cand[:], dslice)
                nc.vector.tensor_mul(acc[:], upd[:], acc[:])
                nc.vector.tensor_add(acc[:], acc[:], dslice)
                nc.scalar.dma_start(out[:, g * 512:(g + 1) * 512], acc[:])


# ---------------- host side ----------------

E3 = ml_dtypes.float8_e3m4
E4 = ml_dtypes.float8_e4m3


def _fp8_scale(w, mx=15.5):
    """Power-of-2 scale landing absmax(w) at ~75% of the fp8 max."""
    absmax = float(np.abs(w).max())
    if absmax == 0.0:
        return 1.0
    return float(2.0 ** np.floor(np.log2(mx * 0.75 / absmax)))


def _gemm_chunks(w, c, nt=None):
    """w [K, N] f32 -> list of [128, 512] fp32 chunks (scaled by c) in
    emission order: n-tile major, k-chunk minor."""
    K, N = w.shape
    nt = N // 512 if nt is None else nt
    kc = K // 128
    out = []
    for n in range(nt):
        for k in range(kc):
            out.append(w[k * 128:(k + 1) * 128, n * 512:(n + 1) * 512] * c)
    return out


def _sbuf_image_T(x, nchunks, dtype=BF16):
    """x [BL, D] -> [128, D] image where S[p, c*128+m] = x[m, 128c+p]."""
    BLl, D = x.shape
    assert D == nchunks * 128 and BLl == BL
    t = x.T.reshape(nchunks, 128, BLl).transpose(1, 0, 2)
    return np.ascontiguousarray(t.reshape(128, D)).astype(dtype)


def _prep_shared(inp):
    """Pack weights/biases/gains (shared across cores). Weights go to fp8
    e3m4 with per-layer power-of-2 scales c; biases are scaled to match, and
    the scale is undone on device (c^2-adjusted eps for normed layers, 1/c
    activation scale for the gate layer)."""
    sh = {}
    c0 = _fp8_scale(inp["w0"])
    c1 = _fp8_scale(inp["w1"])
    c2 = _fp8_scale(inp["w2"])
    ch0 = _fp8_scale(inp["hw0"], mx=240.0)
    ch1 = _fp8_scale(inp["hw1"], mx=240.0)
    cg = _fp8_scale(inp["gw"])

    # flat weight stream in exact consumption order (see _emit); h0/h1
    # regions are e4m3 bytes (DoubleRow), the rest e3m4
    chunks = []
    chunks += [(c, E3) for c in _gemm_chunks(inp["w0"], c0)]
    chunks += [(c, E3) for c in _gemm_chunks(inp["w1"], c1)]
    chunks += [(c, E3) for c in _gemm_chunks(inp["w2"], c2)]
    for g in range(BLOCKS):
        chunks += [(c, E4) for c in _gemm_chunks(inp["hw0"][g], ch0)]
    for g in range(BLOCKS):
        chunks += [(c, E4) for c in _gemm_chunks(inp["hw1"][g], ch1)]
    for g in range(BLOCKS):
        chunks += [(c, E3) for c in _gemm_chunks(inp["gw"][g], cg)]
    assert len(chunks) == N_WCHUNKS
    flat = np.stack([np.asarray(c, F32).astype(dtp).view(np.uint8)
                     for c, dtp in chunks])          # [434, 128, 512] bytes
    flat = flat.transpose(1, 0, 2).reshape(128, N_WCHUNKS * 512).view(E3)
    pad = WTOT - N_WCHUNKS * 512
    sh["wall"] = np.ascontiguousarray(
        np.concatenate([flat, np.zeros((128, pad), E3)], axis=1))

    grows = np.concatenate(
        [inp[k].reshape(-1, 128) for k in ("g0", "g1", "g2", "hg0", "hg1")],
        axis=0).astype(F32)
    extra = np.zeros((N_GROWS - 88, 128), F32)
    for i, c in enumerate((c0, c1, c2, ch0, ch1)):
        extra[i, :] = c * c * EPS
    extra[INVCG_ROW - 88, :] = 1.0 / cg
    sh["gains"] = np.concatenate([grows, extra], axis=0)
    _b = np.concatenate(
        [inp["b0"] * c0, inp["b1"] * c1, inp["b2"] * c2,
         inp["hb0"] * ch0, inp["hb1"] * ch1, inp["gb"] * cg])
    _b = np.concatenate([_b, np.zeros(3 * 8192 - BROW_LEN, _b.dtype)])
    sh["brow"] = _b.reshape(3, 8192).astype(BF16)
    return sh


def kernel(**inputs):
    inputs = {k: np.asarray(v) for k, v in inputs.items()}
    stoch = inputs["stoch"].reshape(B, -1).astype(F32)
    deter = inputs["deter"].astype(F32)
    action = inputs["action"].astype(F32)
    assert deter.shape == (B, DETER) and stoch.shape == (B, STOCH)
    assert action.shape == (B, ACT_D)

    if "nc" not in _CACHE:
        _CACHE["nc"] = _build_nc()
    nc = _CACHE["nc"]

    sh = _prep_shared(inputs)

    in_maps = []
    for c in range(N_CORES):
        s = slice(c * BL, (c + 1) * BL)
        m = dict(sh)
        m["deter"] = np.ascontiguousarray(deter[s])
        m["deterT"] = _sbuf_image_T(deter[s], 32)
        m["deterT8"] = _sbuf_image_T(deter[s], 32, dtype=E4)
        m["stochT"] = _sbuf_image_T(stoch[s], 8)
        m["actT"] = np.ascontiguousarray(action[s].T).astype(F32)
        in_maps.append(m)

    res = run_bass_kernel_spmd(nc, in_maps, core_ids=list(range(N_CORES)))
    return np.concatenate([res.results[c]["out"] for c in range(N_CORES)], axis=0)

